# revision 14
# baseline (speedup 1.0000x reference)
"""Tensor-parallel causal attention (MQA, partial NeoX RoPE) on 8 TRN2 NeuronCores.

Sharding (tensor-parallel over heads, as in the original module):
  core c owns query heads [16c, 16c+16) (rows of Wq), kv head c (rows of Wkv),
  and columns [1024c, 1024(c+1)) of Wo.  Attention is embarrassingly parallel
  per head group; the dense output projection produces per-core partials that
  are combined with an on-device ReduceScatter (token-sharded), chunked along
  the output-feature axis so the collective overlaps the tail of the output
  projection.  The host concatenates the 8 disjoint token shards.

Per-core dataflow (all matmuls bf16, accumulation + softmax math in f32):
  phase 0: constants (identity, causal masks, rope cos/sin from position_ids)
  phase 1: per 512-token chunk: cast hs tile to bf16, PE-transpose into
           hsT [hid, tok]; Q projection (WqT streamed via xbar transpose-DMA
           reads of a bf16 copy of Wq) and K/V projection, evicted through
           fused RoPE into qT/kT (bf16) and vT.
  phase 2: per (batch, head): scoresT[j,i] = kT.T @ qT (no transposes needed
           downstream); exp (scale=1/8 folded in, no max-subtraction -- scores
           are bounded here); multiplicative causal mask on diagonal blocks;
           PV matmul with a ones-column appended to V gives the softmax
           denominator for free; normalize on eviction -> attnT (bf16).
  phase 3: output projection out[t,o] = attnT.T @ WoT, o-chunked; each
           [T, 1024] bf16 partial fires a ReduceScatter; result is cast back
           to f32 into the [T/8, 8192] external output shard.
"""

import math
from dataclasses import dataclass

import numpy as np


# ---------------------------------------------------------------- config

@dataclass(frozen=True)
class Cfg:
    n_cores: int = 8
    T: int = 2048          # total tokens (B*S)
    B: int = 2             # sequences
    HID: int = 8192        # hidden size (= total heads * D)
    MQ: int = 1024         # per-core query dims (16 heads * 64)
    D: int = 64            # head dim
    RD: int = 32           # rotary dims (first RD of each head)
    CHUNK: int = 512       # phase-1 token chunk
    IC: int = 512          # attention i-chunk width
    RSW: int = 1024        # reduce-scatter chunk width along output features

    @property
    def S(self):
        return self.T // self.B

    @property
    def KT(self):
        return self.HID // 128   # k-tiles

    @property
    def MT(self):
        return self.MQ // 128    # per-core q-dim tiles (2 heads per tile)


FULL = Cfg()
SMALL = Cfg(T=256, B=2, HID=1024, MQ=128, CHUNK=128, IC=128, RSW=1024)
MED = Cfg(T=1024, B=2, HID=4096, MQ=512, CHUNK=256, IC=256, RSW=1024)


# ---------------------------------------------------------------- builder

def build_nc(cfg: Cfg, enable_asserts: bool = False, debug: bool = False):
    import concourse.bass as bass
    import concourse.mybir as mybir
    import concourse.tile as tile
    from concourse import bacc
    from concourse.masks import make_identity

    f32 = mybir.dt.float32
    bf16 = mybir.dt.bfloat16
    i32 = mybir.dt.int32

    T, HID, MQ, D, RD = cfg.T, cfg.HID, cfg.MQ, cfg.D, cfg.RD
    B, S = cfg.B, cfg.S
    CHUNK, IC, RSW = cfg.CHUNK, cfg.IC, cfg.RSW
    KT, MT = cfg.KT, cfg.MT
    NCH = T // CHUNK
    TSUB = CHUNK // 128
    HALF = RD // 2                      # 16
    SCALE = 1.0 / math.sqrt(D)
    NJ = 128                            # j-tile width
    N_RS = HID // RSW                   # number of reduce-scatter chunks
    OC_PER_RS = RSW // 512              # 512-wide o-chunks per RS chunk
    TOUT = T // cfg.n_cores             # output rows per core

    nc = bacc.Bacc(
        "TRN2",
        target_bir_lowering=False,
        debug=debug,
        enable_asserts=enable_asserts,
        num_devices=cfg.n_cores,
    )

    hs_ext = nc.dram_tensor("hs", [T, HID], f32, kind="ExternalInput").ap()
    wq_ext = nc.dram_tensor("wq", [MQ, HID], f32, kind="ExternalInput").ap()
    wkv_ext = nc.dram_tensor("wkv", [2 * D, HID], f32, kind="ExternalInput").ap()
    wo_ext = nc.dram_tensor("wo", [HID, MQ], f32, kind="ExternalInput").ap()
    pos_ext = nc.dram_tensor("pos", [1, T], f32, kind="ExternalInput").ap()
    out_ext = nc.dram_tensor("out", [TOUT, HID], f32, kind="ExternalOutput").ap()

    groups = [list(range(cfg.n_cores))]

    with tile.TileContext(nc) as tc:
        with (
            tc.tile_pool(name="const", bufs=1) as const_pool,
            tc.tile_pool(name="persist", bufs=1) as pp,
            tc.tile_pool(name="dram", bufs=1, space="DRAM") as dram,
        ):
            # ---- constants ------------------------------------------------
            identity = const_pool.tile([128, 128], bf16)
            make_identity(nc, identity)

            # causal masks for diagonal blocks: keep where j <= i,
            # i.e. -p + f + delta >= 0 (p = j offset, f = i offset)
            # diagonal blocks have delta = i0 - j0 in {0, -NJ, ..., -(IC-NJ)}
            # (j-tile inside the i-chunk); keep where p <= f + delta
            masks = []
            for di in range(IC // NJ):
                mk = const_pool.tile([128, IC], bf16, name=f"mask{di}")
                nc.gpsimd.memset(mk, 1.0)
                nc.gpsimd.affine_select(
                    out=mk, in_=mk,
                    compare_op=mybir.AluOpType.is_ge,
                    fill=0.0, base=-di * NJ,
                    pattern=[[1, IC]], channel_multiplier=-1,
                )
                masks.append(mk)

            # rope tables: cosT/sinT [HALF, T] f32
            iota_i = const_pool.tile([HALF, 1], i32)
            nc.gpsimd.iota(iota_i, pattern=[[1, 1]], base=0, channel_multiplier=1)
            iota_f = const_pool.tile([HALF, 1], f32)
            nc.vector.tensor_copy(iota_f, iota_i)
            invf = const_pool.tile([HALF, 1], f32)
            nc.scalar.activation(
                invf, iota_f, mybir.ActivationFunctionType.Exp,
                scale=-math.log(10000.0) / HALF,
            )
            sinT = const_pool.tile([HALF, T], f32)
            cosT = const_pool.tile([HALF, T], f32)
            twopi = 2.0 * math.pi
            c1 = 6.28125
            c2 = float(np.float32(twopi - c1))
            c3 = twopi - c1 - float(c2)
            with tc.tile_pool(name="ropetmp", bufs=1) as rtp:
                pos_sb = rtp.tile([1, T], f32)
                nc.sync.dma_start(out=pos_sb, in_=pos_ext)
                posb = rtp.tile([HALF, T], f32)
                nc.gpsimd.partition_broadcast(posb, pos_sb)
                freqT = rtp.tile([HALF, T], f32)
                nc.vector.tensor_scalar_mul(freqT, posb, invf)
                # range-reduce freqs into (-pi, pi] before ScalarE Sin
                # (Cody-Waite cascade, k = trunc/round(x / 2pi))
                kf = rtp.tile([HALF, T], f32)
                nc.vector.tensor_scalar_mul(kf, freqT, 1.0 / twopi)
                ki = rtp.tile([HALF, T], i32)
                nc.vector.tensor_copy(ki, kf)
                nc.vector.tensor_copy(kf, ki)
                red = rtp.tile([HALF, T], f32)
                nc.vector.cody_waite_cascade(red, freqT, kf, c1, c2, c3)
                sarg = rtp.tile([HALF, T], f32)
                nc.vector.add_range_wrap(sarg, red, 0.0, math.pi, twopi)
                nc.scalar.activation(
                    sinT, sarg, mybir.ActivationFunctionType.Sin
                )
                carg = rtp.tile([HALF, T], f32)
                nc.vector.add_range_wrap(carg, red, math.pi / 2, math.pi, twopi)
                nc.scalar.activation(
                    cosT, carg, mybir.ActivationFunctionType.Sin
                )

            # ---- persistent activations ----------------------------------
            qT = pp.tile([128, MT, T], bf16)          # q-dim-major, rope'd
            # k replicated in both partition halves so scores matmuls can
            # align lhsT/rhs base partitions for odd heads
            kT2 = pp.tile([128, T], bf16)
            vT = pp.tile([64, T], bf16)

            # bf16 DRAM copy of wq, k-blocks for fine-grained deps
            NB = max(1, HID // 2048)
            KB = HID // NB
            wq_bf = [
                dram.tile([MQ, KB], bf16, name=f"wq_bf{i}") for i in range(NB)
            ]
            for i in range(NB):
                nc.gpsimd.dma_start(
                    out=wq_bf[i][:, :], in_=wq_ext[:, i * KB:(i + 1) * KB]
                )
            # bf16 DRAM copy of wo (cast later overlaps phases 1-2; emitted
            # here so the DMA queue can start it whenever bandwidth allows --
            # reads of it only happen in phase 3)
            OB = HID // NB
            wo_bf = [
                dram.tile([OB, MQ], bf16, name=f"wo_bf{i}") for i in range(NB)
            ]

            # reduce-scatter buffers
            partials = [
                dram.tile([T, RSW], bf16, name=f"partial{j}") for j in range(N_RS)
            ]
            rs_outs = [
                dram.tile([TOUT, RSW], bf16, name=f"rs_out{j}")
                for j in range(N_RS)
            ]

            # ---- rope eviction helper ------------------------------------
            # Head dims are PERMUTED (host-side weight layout) to
            # [rot1, pass_a, rot2, pass_b] so every engine operand starts at
            # a legal partition offset (0/32/64/96): rot pairs are (d, d+32).
            def rope_evict(rp, psrc, dst, nheads, c0, c1):
                """psrc [64*nheads, w] f32 psum -> dst bf16 with fused rope."""
                w = c1 - c0
                cs = cosT[:, c0:c1]
                sn = sinT[:, c0:c1]
                # base copy casts everything; rot slices overwritten below
                nc.scalar.activation(
                    dst[0:64 * nheads, :], psrc[0:64 * nheads, :],
                    mybir.ActivationFunctionType.Copy,
                )
                for hb in range(0, 64 * nheads, 64):
                    a = psrc[hb:hb + HALF, :]              # rot1 (start 0/64)
                    b = psrc[hb + 32:hb + 32 + HALF, :]    # rot2 (start 32/96)
                    t1 = rp.tile([HALF, w], f32, tag="rt1", name="t1")
                    t2 = rp.tile([HALF, w], f32, tag="rt2", name="t2")
                    nc.vector.tensor_mul(t1, a, cs)
                    nc.vector.tensor_mul(t2, b, sn)
                    nc.vector.tensor_sub(dst[hb:hb + HALF, :], t1, t2)
                    t3 = rp.tile([HALF, w], f32, tag="rt1", name="t3")
                    t4 = rp.tile([HALF, w], f32, tag="rt2", name="t4")
                    nc.vector.tensor_mul(t3, b, cs)
                    nc.vector.tensor_mul(t4, a, sn)
                    nc.vector.tensor_add(dst[hb + 32:hb + 32 + HALF, :], t3, t4)

            # ================= phase 1: projections =======================
            with (
                tc.tile_pool(name="stage", bufs=2) as stage_pool,
                tc.tile_pool(name="hst", bufs=1) as hst_pool,
                tc.tile_pool(name="wqt", bufs=4) as wqt_pool,
                tc.tile_pool(name="rope", bufs=2) as rp,
                tc.tile_pool(name="p1ps", bufs=1, space="PSUM") as ps1,
                tc.tile_pool(name="tpps", bufs=2, space="PSUM") as ps_tp,
            ):
                wkvT = hst_pool.tile([128, KT, 128], bf16)
                # wkv -> wkvT via cast-DMA + PE transpose (once)
                kvstage = stage_pool.tile([128, HID], bf16, tag="stg")
                nc.gpsimd.dma_start(out=kvstage, in_=wkv_ext[:, :])
                for k in range(KT):
                    ptp = ps_tp.tile([128, 128], bf16, tag="tp")
                    nc.tensor.transpose(
                        ptp, kvstage[:, k * 128:(k + 1) * 128], identity
                    )
                    nc.scalar.activation(
                        wkvT[:, k, :], ptp, mybir.ActivationFunctionType.Copy
                    )

                MG = min(4, MT)  # m-tiles per PSUM group
                for c in range(NCH):
                    c0 = c * CHUNK
                    hsT = hst_pool.tile([128, KT, CHUNK], bf16, tag="hsT")
                    # stage hs rows (f32->bf16 cast DMA), PE-transpose to hsT
                    for ts in range(TSUB):
                        stg = stage_pool.tile([128, HID], bf16, tag="stg")
                        r0 = c0 + ts * 128
                        nc.gpsimd.dma_start(out=stg, in_=hs_ext[r0:r0 + 128, :])
                        for k in range(KT):
                            ptp = ps_tp.tile([128, 128], bf16, tag="tp")
                            nc.tensor.transpose(
                                ptp, stg[:, k * 128:(k + 1) * 128], identity
                            )
                            nc.scalar.activation(
                                hsT[:, k, ts * 128:(ts + 1) * 128], ptp,
                                mybir.ActivationFunctionType.Copy,
                            )

                    # Q projection, m-groups of MG
                    for mg in range(MT // MG):
                        MW = MG * 128
                        psq = [
                            ps1.tile([128, CHUNK], f32, tag=f"psq{m}",
                                     bufs=1, name=f"psq{m}")
                            for m in range(MG)
                        ]
                        for k in range(KT):
                            wqt = wqt_pool.tile([128, MW], bf16, tag="wqt")
                            blk = wq_bf[k * 128 // KB]
                            kk = (k * 128) % KB
                            nc.sync.dma_start(
                                out=wqt,
                                in_=blk[mg * MW:(mg + 1) * MW, kk:kk + 128],
                                transpose=True,
                            )
                            for m in range(MG):
                                nc.tensor.matmul(
                                    psq[m][:, :],
                                    lhsT=wqt[:, m * 128:(m + 1) * 128],
                                    rhs=hsT[:, k, :],
                                    start=(k == 0), stop=(k == KT - 1),
                                )
                        for m in range(MG):
                            mt = mg * MG + m
                            rope_evict(rp, psq[m], qT[:, mt, c0:c0 + CHUNK],
                                       2, c0, c0 + CHUNK)

                    # K/V projection for this chunk; k is rope'd into the
                    # lower half of kT2 and replicated to the upper half by a
                    # partition-shifting SBUF->SBUF DMA
                    psk = ps1.tile([64, CHUNK], f32, tag="psk", bufs=1)
                    psv = ps1.tile([64, CHUNK], f32, tag="psv", bufs=1)
                    for k in range(KT):
                        nc.tensor.matmul(
                            psk[:, :], lhsT=wkvT[:, k, 0:64], rhs=hsT[:, k, :],
                            start=(k == 0), stop=(k == KT - 1),
                        )
                        nc.tensor.matmul(
                            psv[:, :], lhsT=wkvT[:, k, 64:128], rhs=hsT[:, k, :],
                            start=(k == 0), stop=(k == KT - 1),
                        )
                    rope_evict(rp, psk, kT2[0:64, c0:c0 + CHUNK],
                               1, c0, c0 + CHUNK)
                    nc.sync.dma_start(
                        out=kT2[64:128, c0:c0 + CHUNK],
                        in_=kT2[0:64, c0:c0 + CHUNK],
                    )
                    nc.vector.tensor_copy(vT[:, c0:c0 + CHUNK], psv[:, :])

            # ================= phase 2: attention =========================
            attnT_ctx = tc.tile_pool(name="attnp", bufs=1)
            ap2 = attnT_ctx.__enter__()
            attnT = ap2.tile([128, MT, T], bf16, name="attnT")
            with (
                tc.tile_pool(name="vext", bufs=1) as vext_pool,
                tc.tile_pool(name="probs", bufs=4) as probs_pool,
                tc.tile_pool(name="nrm", bufs=2) as nrm_pool,
                tc.tile_pool(name="p2ps", bufs=1, space="PSUM") as ps2,
                tc.tile_pool(name="tp2ps", bufs=2, space="PSUM") as ps_tp2,
            ):
                # vext[j-tile] = [128 tokens, D (v dims) + ones column]
                vext = vext_pool.tile([128, T // 128, D + 1], bf16)
                for jt in range(T // 128):
                    ptp = ps_tp2.tile([128, 64], bf16, tag="tpv")
                    nc.tensor.transpose(
                        ptp, vT[:, jt * 128:(jt + 1) * 128], identity[0:64, 0:64]
                    )
                    nc.vector.tensor_copy(vext[:, jt, 0:D], ptp)
                nc.vector.memset(vext[:, :, D:D + 1], 1.0)

                n_heads = MQ // D
                for b in range(B):
                    for h in range(n_heads):
                        mt, hh = h // 2, h % 2
                        for ic in range(S // IC):
                            i0 = b * S + ic * IC
                            qs = qT[hh * D:(hh + 1) * D, mt, i0:i0 + IC]
                            pso = ps2.tile([128, IC], f32, tag="pso", bufs=2)
                            njt = (ic + 1) * (IC // NJ)
                            for jt in range(njt):
                                j0 = b * S + jt * NJ
                                pss = ps2.tile([128, IC], f32, tag="pss", bufs=3)
                                nc.tensor.matmul(
                                    pss[:, :],
                                    lhsT=kT2[hh * D:(hh + 1) * D, j0:j0 + NJ],
                                    rhs=qs,
                                    start=True, stop=True,
                                )
                                pb = probs_pool.tile([128, IC], bf16, tag="pb")
                                nc.scalar.activation(
                                    pb, pss, mybir.ActivationFunctionType.Exp,
                                    scale=SCALE,
                                )
                                dlt = ic * IC - jt * NJ
                                if dlt <= 0:  # diagonal block (j-tile in i-chunk)
                                    nc.vector.tensor_mul(pb, pb, masks[-dlt // NJ])
                                nc.tensor.matmul(
                                    pso[0:D + 1, :],
                                    lhsT=vext[:, (b * S) // 128 + jt, :],
                                    rhs=pb,
                                    start=(jt == 0), stop=(jt == njt - 1),
                                )
                            rc = nrm_pool.tile([1, IC], f32, tag="rc")
                            nc.vector.reciprocal(rc, pso[D:D + 1, :])
                            rcb = nrm_pool.tile([64, IC], f32, tag="rcb")
                            nc.gpsimd.partition_broadcast(rcb, rc)
                            nc.vector.tensor_mul(
                                attnT[hh * D:(hh + 1) * D, mt, i0:i0 + IC],
                                pso[0:D, :], rcb,
                            )

            # ================= phase 3: output projection + RS ============
            # cast wo -> bf16 (DMA only; reads happen below)
            for i in range(NB):
                nc.gpsimd.dma_start(
                    out=wo_bf[i][:, :], in_=wo_ext[i * OB:(i + 1) * OB, :]
                )
            with (
                tc.tile_pool(name="wot", bufs=2) as wot_pool,
                tc.tile_pool(name="pout", bufs=4) as pout_pool,
                tc.tile_pool(name="p3ps", bufs=4, space="PSUM") as ps3,
            ):
                for oc in range(HID // 512):
                    o0 = oc * 512
                    wots = []
                    for a in range(MT):
                        w = wot_pool.tile([128, 512], bf16, tag=f"wot{a}")
                        blk = wo_bf[o0 // OB]
                        oo = o0 % OB
                        nc.sync.dma_start(
                            out=w,
                            in_=blk[oo:oo + 512, a * 128:(a + 1) * 128],
                            transpose=True,
                        )
                        wots.append(w)
                    j = oc // OC_PER_RS
                    jo = (oc % OC_PER_RS) * 512
                    for t in range(T // 128):
                        ps = ps3.tile([128, 512], f32, tag="pso3")
                        for a in range(MT):
                            nc.tensor.matmul(
                                ps[:, :],
                                lhsT=attnT[:, a, t * 128:(t + 1) * 128],
                                rhs=wots[a],
                                start=(a == 0), stop=(a == MT - 1),
                            )
                        ob = pout_pool.tile([128, 512], bf16, tag="ob")
                        nc.scalar.activation(
                            ob, ps, mybir.ActivationFunctionType.Copy
                        )
                        nc.sync.dma_start(
                            out=partials[j][t * 128:(t + 1) * 128, jo:jo + 512],
                            in_=ob,
                        )
                    if oc % OC_PER_RS == OC_PER_RS - 1:
                        nc.gpsimd.collective_compute(
                            "ReduceScatter",
                            mybir.AluOpType.add,
                            ins=[partials[j][:, :].opt()],
                            outs=[rs_outs[j][:, :].opt()],
                            replica_groups=groups,
                        )
                        nc.gpsimd.dma_start(
                            out=out_ext[:, j * RSW:(j + 1) * RSW],
                            in_=rs_outs[j][:, :],
                        )
            attnT_ctx.__exit__(None, None, None)

    nc.compile()
    return nc


# ---------------------------------------------------------------- host side

def shard_inputs(cfg: Cfg, position_ids, hidden_states, Wq, Wkv, Wo):
    """Full inputs -> per-core input maps (slicing/layout/dtype only)."""
    hs = np.ascontiguousarray(np.asarray(hidden_states, dtype=np.float32))
    pos = np.asarray(position_ids).astype(np.float32).reshape(1, cfg.T)
    Wq = np.asarray(Wq, dtype=np.float32)
    Wkv = np.asarray(Wkv, dtype=np.float32)
    Wo = np.asarray(Wo, dtype=np.float32)
    D = cfg.D
    HKV = cfg.n_cores
    half = cfg.RD // 2
    # permuted head-dim order [rot1, pass_a, rot2, pass_b]: rope pairs land
    # at partition offsets (d, d+32), which the engines can address
    perm = np.concatenate([
        np.arange(0, half),
        np.arange(2 * half, 3 * half),
        np.arange(half, 2 * half),
        np.arange(3 * half, D),
    ])
    in_maps = []
    for c in range(cfg.n_cores):
        wq_c = Wq[c * cfg.MQ:(c + 1) * cfg.MQ, :]
        wq_c = np.ascontiguousarray(
            wq_c.reshape(-1, D, cfg.HID)[:, perm, :].reshape(cfg.MQ, cfg.HID)
        )
        wk_c = Wkv[c * D:(c + 1) * D, :][perm, :]
        wv_c = Wkv[HKV * D + c * D:HKV * D + (c + 1) * D, :]
        wkv_c = np.ascontiguousarray(np.concatenate([wk_c, wv_c], axis=0))
        wo_c = np.ascontiguousarray(Wo[:, c * cfg.MQ:(c + 1) * cfg.MQ])
        in_maps.append(
            {"hs": hs, "wq": wq_c, "wkv": wkv_c, "wo": wo_c, "pos": pos}
        )
    return in_maps


_NC_CACHE = {}


def _get_nc(cfg: Cfg):
    if cfg not in _NC_CACHE:
        _NC_CACHE[cfg] = build_nc(cfg)
    return _NC_CACHE[cfg]


def run_on_hw(cfg: Cfg, in_maps, trace=False):
    from concourse.bass_utils import run_bass_kernel_spmd

    nc = _get_nc(cfg)
    res = run_bass_kernel_spmd(
        nc, in_maps, core_ids=list(range(cfg.n_cores)), trace=trace
    )
    out = np.concatenate(
        [np.asarray(res.results[c]["out"], dtype=np.float32)
         for c in range(cfg.n_cores)],
        axis=0,
    )
    return out, res


def kernel(position_ids, hidden_states, Wq, Wkv, Wo, num_seqs):
    cfg = FULL
    in_maps = shard_inputs(cfg, position_ids, hidden_states, Wq, Wkv, Wo)
    out, _ = run_on_hw(cfg, in_maps, trace=False)
    return out
